# revision 27
# baseline (speedup 1.0000x reference)
"""Single-head attention (B=2, S=2048, D=2048, fp32 in/out) on 8 trn2 NeuronCores.

Sharding: sequence-parallel. The 4096 tokens (B*S) are split 512/core; cores
0-3 hold batch 0, cores 4-7 batch 1. Each core computes Q^T, K^T, V for its
512 tokens, K^T/V shards are all-gathered within each 4-core group (one group
per batch), then each core computes scoresT -> exp -> attn@V -> @W_o for
its 512 queries against the full 2048 keys of its batch.

All matmuls run in bf16 (1 cycle/row at N=512, fp32 PSUM accumulation).
Host converts x and weights to bf16; K^T/V shards are produced in bf16 so
the AllGathers move half the bytes of fp32.

per-core phases (each 256 matmuls of K=128, M=128, N=512):
  B: KT_shard(e,t)  = mm(lhsT=W_k[d,e], rhs=xt[d,t])      -> DRAM, AllGather
  C: V_shard(t,e)   = mm(lhsT=xt[d,t],  rhs=W_v[d,e])     -> DRAM, AllGather
  D: QT(e,q)        = mm(lhsT=W_q'[d,e], rhs=xt[d,q])     -> SBUF  (W_q'=W_q/sqrt(D))
  E: scoresT(k,q)   = mm(lhsT=KTg[e,k], rhs=QT[e,q]); exp -> attnT (bf16, direct;
     no PE transposes). rowsums via all-ones matmuls (broadcast on all
     partitions) in short-lived per-kc PSUM tiles from the main mm pool,
     DVE-accumulated in SBUF and emitted one kc late so the PE never waits
     on that kc's exps; all 8 PSUM banks stay in the mm pool (4+4 double
     buffering); reciprocal on DVE.
  F: outT(e,q)      = mm(lhsT=Vg[k,e], rhs=attnT[k,q]); evacuation fused with
     normalization: outT_sb = psum * recip_bcast (DVE)
  G: final(q,d)     = mm(lhsT=outT[e,q], rhs=W_o[e,d]) -> out DRAM (bf16,
     upcast to fp32 on host; halves the tail store, +0.25e-3 rel err)

Weight/V streams load [128,1024] bf16 chunks (2KB lines). C/G outputs are
stored per-PSUM ([128,512]) as each bank evacuates, so the V AllGather can
start before phase C fully ends and the final-store drain tail is one 128KB
DMA instead of four bunched 512KB ones (~1us single-shot + earlier gather;
loop-neutral -- DMA instruction count was A/B'd to a tie). All loads
and stores issue on the sync (SP) queue; the scalar (ACT) queue is reserved
for PSUM evacuations (measured ~18us/iter faster than splitting loads across
both queues — evac ops queueing behind dma_starts delay PSUM recycling and
stall the PE). Exception: phase B's first weight group goes on scalar so the
PE can start ~2us after launch instead of waiting for all 16 x^T loads.

Measured (8-core HW, loop slope at N=1002): ~390-410 us/iter per core without
collectives; cost-model sim: 355us single-shot, PE busy 334us (94% occupancy,
PE roofline 331us at bf16 1 cycle/row). fp8-e4m3 DoubleRow (0.5 cyc/row) was
evaluated and rejected: >=3.4% rel err per fp8 matmul vs the 2e-2 gate.
Also measured and rejected (same-session A/B, each worse or tied):
round-robin loads on sync+scalar (+18us), stores on scalar (+19us bundled
with deeper prefetch), stores on gpsimd/SWDGE (+7us), [128,512] vs [128,1024]
load chunks (tie).
"""
import math
import numpy as np
import ml_dtypes

import concourse.bass as bass
import concourse.mybir as mybir
import concourse.tile as tile
from concourse import bacc
from concourse.bass_utils import run_bass_kernel_spmd

F32 = mybir.dt.float32
BF16 = mybir.dt.bfloat16

D = 2048          # d_model
B = 2
S = 2048
NCORES = 8
GS = 4            # group size (cores per batch)
TOK = 512         # tokens per core
P = 128
NT = D // P       # 16 tiles along d/e
QT_N = TOK // P   # 4 q tiles per core
KC_N = S // 512   # 4 key chunks of 512


def build_attn(n_iters=1, with_collective=True, psum_bufs=None, w_bufs=None, k_bufs=24,
               chunk=1024, gp_stores=False, load_engines=("sync",),
               store_eng="sync", first_scalar=False, rs_pool=True,
               out_bf16=True, ev_bufs=4):
    NSUB = chunk // 512
    NSUP = D // chunk
    if w_bufs is None:
        w_bufs = 24 * 1024 // chunk
    if psum_bufs is None:
        psum_bufs = 8 if rs_pool else 7
    """Build the SPMD attention kernel. n_iters>1 wraps ALL compute phases in
    a timing loop (collectives must be disabled for that)."""
    assert n_iters == 1 or with_collective is False
    nc = bacc.Bacc("TRN2", target_bir_lowering=False, debug=False, num_devices=NCORES)

    xt = nc.dram_tensor("xt", [D, TOK], BF16, kind="ExternalInput")
    wq = nc.dram_tensor("wq", [D, D], BF16, kind="ExternalInput")
    wk = nc.dram_tensor("wk", [D, D], BF16, kind="ExternalInput")
    wv = nc.dram_tensor("wv", [D, D], BF16, kind="ExternalInput")
    wo = nc.dram_tensor("wo", [D, D], BF16, kind="ExternalInput")
    out = nc.dram_tensor("out", [TOK, D], BF16 if out_bf16 else F32,
                         kind="ExternalOutput")

    with tile.TileContext(nc) as tc:
        with (
            tc.tile_pool(name="dram", bufs=1, space="DRAM") as dram,
            tc.tile_pool(name="big", bufs=1) as big,
            tc.tile_pool(name="wide", bufs=w_bufs) as wide,
            tc.tile_pool(name="kpool", bufs=k_bufs) as kpool,
            tc.tile_pool(name="qtpool", bufs=NT) as qtpool,
            tc.tile_pool(name="evpool", bufs=ev_bufs) as evpool,
            tc.tile_pool(name="misc", bufs=1) as misc,
            tc.tile_pool(name="ps", bufs=psum_bufs, space="PSUM") as ps,
            tc.tile_pool(name="ps_rs", bufs=1, space="PSUM") as ps_rs,
        ):
            kt_shard = dram.tile([D, TOK], BF16)
            v_shard = dram.tile([TOK, D], BF16)
            kt_g = dram.tile([GS * D, TOK], BF16)    # [s*D + e, k_local]
            v_g = dram.tile([GS * TOK, D], BF16)     # [k, e]

            xt_sb = [big.tile([P, TOK], BF16, name=f"xt{i}") for i in range(NT)]
            attnT_sb = [big.tile([P, TOK], BF16, name=f"attnT{i}") for i in range(NT)]

            ones_sb = misc.tile([P, P], BF16)
            nc.gpsimd.memset(ones_sb[:], 1.0)
            recip_bcast = misc.tile([P, TOK], F32)
            rs_accum = misc.tile([P, TOK], F32, name="rs_accum") if rs_pool else None

            _eng_i = [0]
            _engines = [getattr(nc, e) for e in load_engines]

            def LD(dst, src_ap, eng=None):
                e = eng if eng is not None else _engines[_eng_i[0] % len(_engines)]
                _eng_i[0] += 1
                e.dma_start(dst, src_ap)

            st_eng = nc.gpsimd if gp_stores else getattr(nc, store_eng)

            def ST(dst_ap, src):
                st_eng.dma_start(dst_ap, src)

            # ---- x^T and phase B's first weight group load interleaved
            # across both queues in dt order (~3MB each, ~10.5us): the PE
            # needs xt[dt]+wk[dt] pairs progressively and starts ~2us in.
            wts_b0 = []
            for dt in range(NT):
                t = wide.tile([P, chunk], BF16, tag="wide", name=f"pb0_{dt}")
                qa, qb = (nc.scalar, nc.sync) if dt % 2 == 0 else \
                         (nc.sync, nc.scalar)
                qa.dma_start(t[:], wk[dt * P:(dt + 1) * P, 0:chunk])
                qb.dma_start(xt_sb[dt][:], xt[dt * P:(dt + 1) * P, :])
                wts_b0.append(t)

            def wide_group(pfx, load_fn, eng=None):
                """Load 16 (128,chunk) chunks via round-robin engines."""
                ts = []
                for i in range(NT):
                    t = wide.tile([P, chunk], BF16, tag="wide", name=f"{pfx}{i}")
                    LD(t[:], load_fn(i), eng)
                    ts.append(t)
                return ts

            def proj_to_T(w_dram, dest_cb, pfx, preloaded=None):
                """QT/KT-style projection: out[e,t] = sum_d W[d,e]*xt[d,t].
                16-deep same-bank accumulation chains, [128,1024] loads."""
                for eg2 in range(NSUP):
                    if eg2 == 0 and preloaded is not None:
                        wts = preloaded
                    else:
                        wts = wide_group(pfx, lambda dt: w_dram[
                            dt * P:(dt + 1) * P,
                            eg2 * chunk:(eg2 + 1) * chunk])
                    for sub in range(NSUB):
                        eg = eg2 * NSUB + sub
                        psums = [ps.tile([P, 512], F32, tag="mm", name=f"{pfx}p{i}")
                                 for i in range(4)]
                        for half in range(2):
                            for j in range(4):
                                for dt8 in range(8):
                                    dt = half * 8 + dt8
                                    nc.tensor.matmul(
                                        psums[j][:],
                                        wts[dt][:, sub * 512 + j * P:
                                                sub * 512 + (j + 1) * P],
                                        xt_sb[dt][:],
                                        start=(dt == 0), stop=(dt == NT - 1))
                        for j in range(4):
                            dest_cb(eg * 4 + j, psums[j])

            def b_dest(et, psum):
                ev = evpool.tile([P, 512], BF16, tag="ev", name="evb")
                nc.scalar.copy(ev[:], psum[:])
                ST(kt_shard[et * P:(et + 1) * P, :], ev[:])

            def phase_c():
                for ec2 in range(NSUP):
                    wvs = wide_group("cw", lambda dt: wv[
                        dt * P:(dt + 1) * P, ec2 * chunk:(ec2 + 1) * chunk])
                    for sub in range(NSUB):
                        ec = ec2 * NSUB + sub
                        psums = [ps.tile([P, 512], F32, tag="mm", name=f"pvp{i}")
                                 for i in range(4)]
                        for half in range(2):
                            for tt in range(QT_N):
                                for dt8 in range(8):
                                    dt = half * 8 + dt8
                                    nc.tensor.matmul(
                                        psums[tt][:],
                                        xt_sb[dt][:, tt * P:(tt + 1) * P],
                                        wvs[dt][:, sub * 512:(sub + 1) * 512],
                                        start=(dt == 0), stop=(dt == NT - 1))
                        for tt in range(QT_N):
                            ev = evpool.tile([P, 512], BF16, tag="ev",
                                             name="evc")
                            nc.scalar.copy(ev[:], psums[tt][:])
                            ST(v_shard[tt * P:(tt + 1) * P,
                                       ec * 512:(ec + 1) * 512], ev[:])

            def phases_defg():
                # ---- phase D: QT (tiles share slots with outT via tag)
                qt_sb = [qtpool.tile([P, TOK], BF16, tag="qo", name=f"qt{i}")
                         for i in range(NT)]

                def d_dest(et, psum):
                    nc.scalar.copy(qt_sb[et][:], psum[:])
                proj_to_T(wq, d_dest, "pd")

                # ---- phase E: scoresT -> exp -> attnT (no transposes)
                # rowsums rs[p,q] = sum_k attnT[k,q] via all-ones matmuls.
                # rs_pool mode: per-kc short-lived psum (frees a PSUM bank for
                # the mm pool -> 8-bank 4+4 double buffering), DVE-accumulated
                # in SBUF; each kc's ones-mms are emitted one kc late so they
                # never make the PE wait on that kc's exps.
                def rs_mms(kc):
                    rp = ps.tile([P, TOK], F32, tag="mm", name=f"rsp{kc}")
                    for j in range(4):
                        nc.tensor.matmul(
                            rp[:], ones_sb[:], attnT_sb[kc * 4 + j][:],
                            start=(j == 0), stop=(j == 3))
                    if kc == 0:
                        nc.vector.tensor_copy(rs_accum[:], rp[:])
                    else:
                        nc.vector.tensor_add(rs_accum[:], rs_accum[:], rp[:])

                for kc in range(KC_N):
                    kts = []
                    for et in range(NT):
                        t = kpool.tile([P, 512], BF16, tag="kt", name=f"ek{et}")
                        LD(t[:], kt_g[kc * D + et * P: kc * D + (et + 1) * P, :])
                        kts.append(t)
                    psums = [ps.tile([P, 512], F32, tag="mm", name=f"pep{i}")
                             for i in range(4)]
                    for half in range(2):
                        for j in range(4):
                            for et8 in range(8):
                                et = half * 8 + et8
                                nc.tensor.matmul(
                                    psums[j][:],
                                    kts[et][:, j * P:(j + 1) * P],
                                    qt_sb[et][:],
                                    start=(et == 0), stop=(et == NT - 1))
                    for j in range(4):
                        nc.scalar.activation(
                            attnT_sb[kc * 4 + j][:], psums[j][:],
                            mybir.ActivationFunctionType.Exp)
                    if rs_pool and kc > 0:
                        rs_mms(kc - 1)

                if rs_pool:
                    rs_mms(KC_N - 1)
                    nc.vector.reciprocal(recip_bcast[:], rs_accum[:])
                else:
                    rs_ps = ps_rs.tile([P, TOK], F32, name="rs_ps")
                    for t in range(NT):
                        nc.tensor.matmul(
                            rs_ps[:], ones_sb[:], attnT_sb[t][:],
                            start=(t == 0), stop=(t == NT - 1))
                    nc.vector.reciprocal(recip_bcast[:], rs_ps[:])

                # ---- phase F: outT (slots freed by qt after phase E)
                outT_sb = [qtpool.tile([P, TOK], BF16, tag="qo", name=f"outT{i}")
                           for i in range(NT)]
                for eg2 in range(NSUP):
                    vts = wide_group("fv", lambda kt: v_g[
                        kt * P:(kt + 1) * P, eg2 * chunk:(eg2 + 1) * chunk])
                    for sub in range(NSUB):
                        eg = eg2 * NSUB + sub
                        psums = [ps.tile([P, 512], F32, tag="mm", name=f"pfp{i}")
                                 for i in range(4)]
                        for half in range(2):
                            for j in range(4):
                                for kt8 in range(8):
                                    kt = half * 8 + kt8
                                    nc.tensor.matmul(
                                        psums[j][:],
                                        vts[kt][:, sub * 512 + j * P:
                                                sub * 512 + (j + 1) * P],
                                        attnT_sb[kt][:],
                                        start=(kt == 0), stop=(kt == NT - 1))
                        for j in range(4):
                            nc.vector.tensor_mul(
                                outT_sb[eg * 4 + j][:], psums[j][:],
                                recip_bcast[:])

                # ---- phase G: final
                for dc2 in range(NSUP):
                    wos = wide_group("gw", lambda et: wo[
                        et * P:(et + 1) * P, dc2 * chunk:(dc2 + 1) * chunk])
                    for sub in range(NSUB):
                        dc = dc2 * NSUB + sub
                        psums = [ps.tile([P, 512], F32, tag="mm", name=f"pgp{i}")
                                 for i in range(4)]
                        for half in range(2):
                            for qt in range(QT_N):
                                for et8 in range(8):
                                    et = half * 8 + et8
                                    nc.tensor.matmul(
                                        psums[qt][:],
                                        outT_sb[et][:, qt * P:(qt + 1) * P],
                                        wos[et][:, sub * 512:(sub + 1) * 512],
                                        start=(et == 0), stop=(et == NT - 1))
                        for qt in range(QT_N):
                            evf = evpool.tile([P, 512],
                                              BF16 if out_bf16 else F32,
                                              tag="evf", name="evf")
                            nc.scalar.copy(evf[:], psums[qt][:])
                            ST(out[qt * P:(qt + 1) * P,
                                   dc * 512:(dc + 1) * 512], evf[:])

            def whole_body(first=False):
                proj_to_T(wk, b_dest, "pb",
                          preloaded=wts_b0 if first else None)
                if with_collective in (True, "k"):
                    nc.gpsimd.collective_compute(
                        "AllGather", mybir.AluOpType.bypass,
                        replica_groups=[[0, 1, 2, 3], [4, 5, 6, 7]],
                        ins=[kt_shard[:].opt()], outs=[kt_g[:].opt()],
                    )
                phase_c()
                if with_collective in (True, "v"):
                    nc.gpsimd.collective_compute(
                        "AllGather", mybir.AluOpType.bypass,
                        replica_groups=[[0, 1, 2, 3], [4, 5, 6, 7]],
                        ins=[v_shard[:].opt()], outs=[v_g[:].opt()],
                    )
                phases_defg()

            if n_iters == 1:
                whole_body(first=True)
            else:
                with tc.For_i(0, n_iters, 1):
                    whole_body()

    nc.compile()
    return nc


_CACHED = {}


def _get_nc():
    if "nc" not in _CACHED:
        _CACHED["nc"] = build_attn()
    return _CACHED["nc"]


def _make_in_maps(inputs):
    x = np.asarray(inputs["x"], np.float32)
    W_q = np.asarray(inputs["W_q"], np.float32)
    W_k = np.asarray(inputs["W_k"], np.float32)
    W_v = np.asarray(inputs["W_v"], np.float32)
    W_o = np.asarray(inputs["W_o"], np.float32)

    bf = ml_dtypes.bfloat16
    scale = np.float32(1.0 / math.sqrt(D))
    wq_s = np.ascontiguousarray((W_q * scale).astype(bf))
    wk_c = np.ascontiguousarray(W_k.astype(bf))
    wv_c = np.ascontiguousarray(W_v.astype(bf))
    wo_c = np.ascontiguousarray(W_o.astype(bf))

    toks = x.reshape(B * S, D)              # (4096, 2048)
    xt_full = np.ascontiguousarray(toks.T.astype(bf))  # (2048, 4096)

    in_maps = []
    for c in range(NCORES):
        in_maps.append({
            "xt": np.ascontiguousarray(xt_full[:, c * TOK:(c + 1) * TOK]),
            "wq": wq_s, "wk": wk_c, "wv": wv_c, "wo": wo_c,
        })
    return in_maps


def kernel(x, W_q, W_k, W_v, W_o):
    in_maps = _make_in_maps(dict(x=x, W_q=W_q, W_k=W_k, W_v=W_v, W_o=W_o))
    nc = _get_nc()
    res = run_bass_kernel_spmd(nc, in_maps, core_ids=list(range(NCORES)))
    rows = np.concatenate(
        [np.asarray(res.results[c]["out"], dtype=np.float32)
         for c in range(NCORES)], axis=0)
    return rows.reshape(B, S, D)
